# revision 2
# baseline (speedup 1.0000x reference)
"""GRU cell kernel for Trainium2, 8-core data-parallel, single dispatch.

Strategy
--------
Data-parallel on batch across 8 cores; each core processes its full
2048-row shard in ONE NEFF dispatch (v1 used two rounds of 1024 —
paying the DMA ramp, HAM re-warm and weight reload twice).  All
on-chip compute happens in *transposed space* ([hidden, batch]) so
every matmul contraction lands on SBUF partitions with no on-device
transposes:

    r^T = sigmoid(W_r @ x^T + U_r @ h^T + b_r)
    u^T = sigmoid(W_u @ x^T + U_u @ h^T + b_u)
    c^T = tanh   (W   @ x^T + U  @ (h.r)^T + b_c)
    o^T = h^T + u^T * (c^T - h^T)

Matmuls run in bf16.  The 2048-wide batch gives 4 moving slices of 512
per stationary weight tile (v1 had 2): the first matmul after a weight
swap stalls until the previous one drains (~150 ns observed), so
fewer swaps per MM directly raises tensor-engine occupancy.

SBUF budget (per partition, ~208 KiB usable): weights 96K + x 32K +
h 32K + hr 16K + pools ~20K.  hr is stored fp8_e4m3 (moving operand
only; measured end-to-end rel err 7.5e-3 vs the 2e-2 budget) and the
output chain runs bf16 in-place in the c tile, else this does not fit.

DMA rules inherited from v1: this toolchain's DMA descriptors encode
exactly ONE sync wait, so no load DMA may target a recycled tile slot
(loads carry only queue-FIFO waits -> every DMA'd tile gets its own
slot), and the 8 output stores ride 8 distinct SWDGE queues so their
single RAW wait fits.  Load order is interleaved (x0, h0, then gate-0
R weights, then the rest) so the PE starts ~4 us in.
"""

import sys

sys.path.insert(0, "/opt/trn_rl_repo")

import numpy as np
import ml_dtypes
from contextlib import ExitStack

import concourse.bass as bass
import concourse.bacc as bacc
import concourse.mybir as mybir
from concourse import tile
from concourse.bass_utils import run_bass_kernel_spmd

BF16 = mybir.dt.bfloat16
FP8 = mybir.dt.float8e4
F32 = mybir.dt.float32
AF = mybir.ActivationFunctionType

N_CORES = 8
B = 16384
D = 1024  # IN == H
B_SHARD = B // N_CORES  # 2048 rows per core, single dispatch
BW = 512  # matmul moving width (one fp32 PSUM bank)


def build_nc(d=D, b_shard=B_SHARD, bw=BW):
    """Build the SPMD per-core Bass program.

    Packed weight order: 0=W_r, 1=U_r, 2=W_u, 3=U_u, 4=W, 5=U.
    Bias columns: [r: 0..nh) [u: nh..2nh) [c: 2nh..3nh).
    """
    nk = d // 128
    nh = d // 128
    nb = b_shard // bw

    nc = bacc.Bacc("TRN2", target_bir_lowering=False)
    xt = nc.dram_tensor("xt", [d, b_shard], BF16, kind="ExternalInput")
    ht = nc.dram_tensor("ht", [d, b_shard], BF16, kind="ExternalInput")
    wts = nc.dram_tensor("wts", [6, nh, nk, 128, 128], BF16, kind="ExternalInput")
    bias = nc.dram_tensor("bias", [128, 3 * nh], F32, kind="ExternalInput")
    out = nc.dram_tensor("out", [d, b_shard], BF16, kind="ExternalOutput")

    with tile.TileContext(nc) as tc, ExitStack() as ctx:
        xp = ctx.enter_context(tc.tile_pool(name="xp", bufs=nk))
        hp = ctx.enter_context(tc.tile_pool(name="hp", bufs=nk))
        hrp = ctx.enter_context(tc.tile_pool(name="hrp", bufs=nh))
        rp = ctx.enter_context(tc.tile_pool(name="rp", bufs=2))
        up = ctx.enter_context(tc.tile_pool(name="up", bufs=2))
        cp = ctx.enter_context(tc.tile_pool(name="cp", bufs=3))
        # every weight tile gets its own slot: no DMA slot reuse anywhere
        wp = ctx.enter_context(tc.tile_pool(name="wp", bufs=6 * nh * nk))
        bp = ctx.enter_context(tc.tile_pool(name="bp", bufs=1))
        pp = ctx.enter_context(tc.tile_pool(name="pp", bufs=8, space="PSUM"))

        btile = bp.tile([128, 3 * nh], F32, name="btile")
        nc.sync.dma_start(btile, bias[:, :])

        # Weight tiles are DMA'd lazily in compute order below; allocate the
        # slot map up front so issue order can differ from allocation order.
        wtiles = {}

        def load_w(mat, j, k):
            t = wp.tile([128, 128], BF16, name="wtile")
            nc.sync.dma_start(t, wts[mat, j, k, :, :])
            wtiles[(mat, j, k)] = t

        # --- load order: x0, h0, gate-0 R weights, rest of x/h, rest of Ws
        xts, hts = [None] * nk, [None] * nk

        def load_x(k):
            xts[k] = xp.tile([128, b_shard], BF16, name="xtile")
            nc.sync.dma_start(xts[k], xt[k * 128 : (k + 1) * 128, :])

        def load_h(k):
            hts[k] = hp.tile([128, b_shard], BF16, name="htile")
            nc.sync.dma_start(hts[k], ht[k * 128 : (k + 1) * 128, :])

        load_x(0)
        load_w(0, 0, 0)
        load_h(0)
        for k in range(1, nk):
            load_w(0, 0, k)
        for k in range(nk):
            load_w(1, 0, k)
        for k in range(1, nk):
            load_x(k)
            load_h(k)
        for j in range(1, nh):
            for mat in (0, 1):
                for k in range(nk):
                    load_w(mat, j, k)
        for mat in (2, 3, 4, 5):
            for j in range(nh):
                for k in range(nk):
                    load_w(mat, j, k)

        def gate_matmuls(j, mat_x, mov_x, mat_h, mov_h):
            """Accumulate x-part + h-part for gate tile j into nb PSUM banks."""
            ps = [pp.tile([128, bw], F32, name="ps") for _ in range(nb)]
            for mi, (mat, mov) in enumerate(((mat_x, mov_x), (mat_h, mov_h))):
                for k in range(nk):
                    lhsT = wtiles[(mat, j, k)]
                    for b in range(nb):
                        nc.tensor.matmul(
                            ps[b],
                            lhsT,
                            mov[k][:, b * bw : (b + 1) * bw],
                            start=(mi == 0 and k == 0),
                            stop=(mi == 1 and k == nk - 1),
                        )
            return ps

        # R phase: r = sigmoid(...); hr = h * r in fp8 (feeds the c matmuls)
        hrs = []
        for j in range(nh):
            ps = gate_matmuls(j, 0, xts, 1, hts)
            rtile = rp.tile([128, b_shard], BF16, name="rtile")
            for b in range(nb):
                nc.scalar.activation(
                    rtile[:, b * bw : (b + 1) * bw], ps[b], AF.Sigmoid,
                    bias=btile[:, j : j + 1],
                )
            hrtile = hrp.tile([128, b_shard], FP8, name="hrtile")
            nc.vector.tensor_mul(hrtile, hts[j], rtile)
            hrs.append(hrtile)

        # U+C fused per j, with the out chain running in-place in ctile
        for j in range(nh):
            ps = gate_matmuls(j, 2, xts, 3, hts)
            util = up.tile([128, b_shard], BF16, name="utile")
            for b in range(nb):
                nc.scalar.activation(
                    util[:, b * bw : (b + 1) * bw], ps[b], AF.Sigmoid,
                    bias=btile[:, nh + j : nh + j + 1],
                )
            ps = gate_matmuls(j, 4, xts, 5, hrs)
            ctile = cp.tile([128, b_shard], BF16, name="ctile")
            for b in range(nb):
                nc.scalar.activation(
                    ctile[:, b * bw : (b + 1) * bw], ps[b], AF.Tanh,
                    bias=btile[:, 2 * nh + j : 2 * nh + j + 1],
                )
            # o = h + u*(c - h), computed in place in ctile (bf16)
            nc.vector.tensor_sub(ctile, ctile, hts[j])
            nc.vector.tensor_mul(ctile, util, ctile)
            nc.vector.tensor_add(ctile, ctile, hts[j])
            # SWDGE: 8 stores over 8 SW queues -> no queue backpressure
            # wait, so the single RAW wait fits the descriptor.
            nc.gpsimd.dma_start(out[j * 128 : (j + 1) * 128, :], ctile)

    # Bacc lowering: splits multi-wait sync into InstEventSemaphore ops
    # (hardware allows one wait per instruction), allocates registers, etc.
    nc.compile()
    return nc


def pack_inputs(inputs, d=D, b_shard=B_SHARD, n_shards=N_CORES):
    """Host-side shard + transpose + cast. Returns per-shard input maps."""
    nk = d // 128
    nh = d // 128
    x = np.asarray(inputs["x_t"], np.float32)
    h = np.asarray(inputs["h_prev"], np.float32)

    mats = [inputs["W_r"], inputs["U_r"], inputs["W_u"], inputs["U_u"],
            inputs["W"], inputs["U"]]
    wts = np.empty((6, nh, nk, 128, 128), ml_dtypes.bfloat16)
    for i, m in enumerate(mats):
        mt = np.asarray(m, np.float32).T.astype(ml_dtypes.bfloat16)  # [in, out]
        # wts[i, j, k][p, m] = M.T[k*128+p, j*128+m]
        wts[i] = mt.reshape(nk, 128, nh, 128).transpose(2, 0, 1, 3)

    b_r = np.asarray(inputs["b_Wr"], np.float32) + np.asarray(inputs["b_Ur"], np.float32)
    b_u = np.asarray(inputs["b_Wu"], np.float32) + np.asarray(inputs["b_Uu"], np.float32)
    b_c = np.asarray(inputs["b_W"], np.float32) + np.asarray(inputs["b_U"], np.float32)
    bias = np.concatenate(
        [bb.reshape(nh, 128).T for bb in (b_r, b_u, b_c)], axis=1
    ).astype(np.float32)  # [128, 3*nh]

    in_maps = []
    for s in range(n_shards):
        rows = slice(s * b_shard, (s + 1) * b_shard)
        xT = np.ascontiguousarray(x[rows].T).astype(ml_dtypes.bfloat16)
        hT = np.ascontiguousarray(h[rows].T).astype(ml_dtypes.bfloat16)
        in_maps.append({"xt": xT, "ht": hT, "wts": wts, "bias": bias})
    return in_maps


_NC_CACHE = {}


def _get_nc():
    if "nc" not in _NC_CACHE:
        _NC_CACHE["nc"] = build_nc()
    return _NC_CACHE["nc"]


def _run(inputs, **spmd_kwargs):
    nc = _get_nc()
    in_maps = pack_inputs(inputs)
    res = run_bass_kernel_spmd(nc, in_maps, list(range(N_CORES)), **spmd_kwargs)
    out = np.empty((B, D), np.float32)
    for c in range(N_CORES):
        out[c * B_SHARD : (c + 1) * B_SHARD, :] = (
            res.results[c]["out"].astype(np.float32).T
        )
    return out, [res]


def kernel(**inputs):
    out, _ = _run(inputs)
    return out


# revision 3
# speedup vs baseline: 1.1877x; 1.1877x over previous
"""GRU cell kernel for Trainium2, 8-core data-parallel, single dispatch.

Strategy
--------
Data-parallel on batch across 8 cores; each core processes its full
2048-row shard in ONE NEFF dispatch.  All on-chip compute happens in
*transposed space* ([hidden, batch]) so every matmul contraction lands
on SBUF partitions with no on-device transposes:

    r^T = sigmoid(W_r @ x^T + U_r @ h^T + b_r)
    u^T = sigmoid(W_u @ x^T + U_u @ h^T + b_u)
    c^T = tanh   (W   @ x^T + U  @ (h.r)^T + b_c)
    o^T = h^T + u^T * (c^T - h^T)

Matmuls run in bf16.  The 2048-wide batch gives 4 moving slices of 512
per stationary weight tile, which keeps the tensor engine at its
~216 ns/MM back-to-back cadence (weight-swap drain stalls amortize).

DMA conveyor: the HWDGE rings dispatch ~1 DMA instruction per ~600 ns
regardless of size, so per-128x128-tile weight loads (384 of them)
rate-limit the whole kernel (measured: PE catches the weight stream
and idles 47 us).  v3 loads weights as 48 contiguous 256 KiB slabs
(one per (matrix, output-tile), host-side packed) on the Sync ring,
and puts x on the Scalar ring so the two conveyors run in parallel.
The R phase consumes k-tiles in arrival order.

SBUF budget (per partition, ~208 KiB usable): weights 96K + x 32K +
h 32K + hr 16K + pools ~20K.  hr is stored fp8_e4m3 (moving operand
only) and the output chain runs bf16 in-place in the c tile; measured
end-to-end rel err 7.5e-3 vs the 2e-2 budget.

DMA rules: descriptors encode exactly ONE sync wait, so no load DMA
may target a recycled tile slot (loads carry only queue-FIFO waits ->
every DMA'd tile gets its own slot), and the 8 output stores ride 8
distinct SWDGE queues so their single RAW wait fits.
"""

import sys

sys.path.insert(0, "/opt/trn_rl_repo")

import numpy as np
import ml_dtypes
from contextlib import ExitStack

import concourse.bass as bass
import concourse.bacc as bacc
import concourse.mybir as mybir
from concourse import tile
from concourse.bass_utils import run_bass_kernel_spmd

BF16 = mybir.dt.bfloat16
FP8 = mybir.dt.float8e4
F32 = mybir.dt.float32
AF = mybir.ActivationFunctionType

N_CORES = 8
B = 16384
D = 1024  # IN == H
B_SHARD = B // N_CORES  # 2048 rows per core, single dispatch
BW = 512  # matmul moving width (one fp32 PSUM bank)

# k-tile consumption order matched to the two-ring arrival interleave
KORD = [0, 4, 1, 5, 2, 6, 3, 7]


def build_nc(d=D, b_shard=B_SHARD, bw=BW):
    """Build the SPMD per-core Bass program.

    Packed weight order: 0=W_r, 1=U_r, 2=W_u, 3=U_u, 4=W, 5=U.
    Bias columns: [r: 0..nh) [u: nh..2nh) [c: 2nh..3nh).
    """
    nk = d // 128
    nh = d // 128
    nb = b_shard // bw

    nc = bacc.Bacc("TRN2", target_bir_lowering=False)
    xt = nc.dram_tensor("xt", [d, b_shard], BF16, kind="ExternalInput")
    ht = nc.dram_tensor("ht", [d, b_shard], BF16, kind="ExternalInput")
    # weight slab (mat, j): [128 partitions, nk*128] contiguous
    wts = nc.dram_tensor("wts", [6, nh, 128, nk * 128], BF16, kind="ExternalInput")
    bias = nc.dram_tensor("bias", [128, 3 * nh], F32, kind="ExternalInput")
    out = nc.dram_tensor("out", [d, b_shard], BF16, kind="ExternalOutput")

    with tile.TileContext(nc) as tc, ExitStack() as ctx:
        xp = ctx.enter_context(tc.tile_pool(name="xp", bufs=nk))
        hp = ctx.enter_context(tc.tile_pool(name="hp", bufs=nk))
        hrp = ctx.enter_context(tc.tile_pool(name="hrp", bufs=nh))
        rp = ctx.enter_context(tc.tile_pool(name="rp", bufs=2))
        up = ctx.enter_context(tc.tile_pool(name="up", bufs=2))
        cp = ctx.enter_context(tc.tile_pool(name="cp", bufs=3))
        # every weight slab gets its own slot: no DMA slot reuse anywhere
        wp = ctx.enter_context(tc.tile_pool(name="wp", bufs=6 * nh))
        bp = ctx.enter_context(tc.tile_pool(name="bp", bufs=1))
        pp = ctx.enter_context(tc.tile_pool(name="pp", bufs=8, space="PSUM"))

        wslabs = {}

        def load_w(mat, j):
            t = wp.tile([128, nk * 128], BF16, name="wslab")
            nc.sync.dma_start(t, wts[mat, j, :, :])
            wslabs[(mat, j)] = t

        xts, hts = [None] * nk, [None] * nk

        def load_x(k):
            xts[k] = xp.tile([128, b_shard], BF16, name="xtile")
            nc.scalar.dma_start(xts[k], xt[k * 128 : (k + 1) * 128, :])

        def load_h(k):
            hts[k] = hp.tile([128, b_shard], BF16, name="htile")
            nc.sync.dma_start(hts[k], ht[k * 128 : (k + 1) * 128, :])

        # scalar ring: bias + all of x.  sync ring: R-phase weight slabs
        # interleaved with h, then the UC-phase slabs in consumption order.
        btile = bp.tile([128, 3 * nh], F32, name="btile")
        nc.scalar.dma_start(btile, bias[:, :])
        for k in range(nk):
            load_x(k)
        load_w(0, 0)
        load_h(0)
        load_h(1)
        load_w(1, 0)
        for k in range(2, nk):
            load_h(k)
        for j in range(1, nh):
            load_w(0, j)
            load_w(1, j)
        for j in range(nh):
            for mat in (2, 3, 4, 5):
                load_w(mat, j)

        def gate_matmuls(j, mat_x, mov_x, mat_h, mov_h):
            """Accumulate x-part + h-part for gate tile j into nb PSUM banks."""
            ps = [pp.tile([128, bw], F32, name="ps") for _ in range(nb)]
            for mi, (mat, mov) in enumerate(((mat_x, mov_x), (mat_h, mov_h))):
                slab = wslabs[(mat, j)]
                for ki, k in enumerate(KORD):
                    lhsT = slab[:, k * 128 : (k + 1) * 128]
                    for b in range(nb):
                        nc.tensor.matmul(
                            ps[b],
                            lhsT,
                            mov[k][:, b * bw : (b + 1) * bw],
                            start=(mi == 0 and ki == 0),
                            stop=(mi == 1 and ki == nk - 1),
                        )
            return ps

        # R phase: r = sigmoid(...); hr = h * r in fp8 (feeds the c matmuls)
        hrs = []
        for j in range(nh):
            ps = gate_matmuls(j, 0, xts, 1, hts)
            rtile = rp.tile([128, b_shard], BF16, name="rtile")
            for b in range(nb):
                nc.scalar.activation(
                    rtile[:, b * bw : (b + 1) * bw], ps[b], AF.Sigmoid,
                    bias=btile[:, j : j + 1],
                )
            hrtile = hrp.tile([128, b_shard], FP8, name="hrtile")
            nc.vector.tensor_mul(hrtile, hts[j], rtile)
            hrs.append(hrtile)

        # U+C fused per j, out chain in-place in ctile, chunked per bank so
        # the tail (ACT -> DVE -> store) pipelines at 512 granularity.
        for j in range(nh):
            ps = gate_matmuls(j, 2, xts, 3, hts)
            util = up.tile([128, b_shard], BF16, name="utile")
            for b in range(nb):
                nc.scalar.activation(
                    util[:, b * bw : (b + 1) * bw], ps[b], AF.Sigmoid,
                    bias=btile[:, nh + j : nh + j + 1],
                )
            ps = gate_matmuls(j, 4, xts, 5, hrs)
            ctile = cp.tile([128, b_shard], BF16, name="ctile")
            for b in range(nb):
                s = slice(b * bw, (b + 1) * bw)
                nc.scalar.activation(
                    ctile[:, s], ps[b], AF.Tanh,
                    bias=btile[:, 2 * nh + j : 2 * nh + j + 1],
                )
                # o = h + u*(c - h), computed in place in ctile (bf16)
                nc.vector.tensor_sub(ctile[:, s], ctile[:, s], hts[j][:, s])
                nc.vector.tensor_mul(ctile[:, s], util[:, s], ctile[:, s])
                nc.vector.tensor_add(ctile[:, s], ctile[:, s], hts[j][:, s])
            # SWDGE: 8 stores over 8 SW queues -> no queue backpressure
            # wait, so the single RAW wait fits the descriptor.
            nc.gpsimd.dma_start(out[j * 128 : (j + 1) * 128, :], ctile)

    # Bacc lowering: splits multi-wait sync into InstEventSemaphore ops
    # (hardware allows one wait per instruction), allocates registers, etc.
    nc.compile()
    return nc


def pack_inputs(inputs, d=D, b_shard=B_SHARD, n_shards=N_CORES):
    """Host-side shard + transpose + cast. Returns per-shard input maps."""
    nk = d // 128
    nh = d // 128
    x = np.asarray(inputs["x_t"], np.float32)
    h = np.asarray(inputs["h_prev"], np.float32)

    mats = [inputs["W_r"], inputs["U_r"], inputs["W_u"], inputs["U_u"],
            inputs["W"], inputs["U"]]
    wts = np.empty((6, nh, 128, nk * 128), ml_dtypes.bfloat16)
    for i, m in enumerate(mats):
        mt = np.asarray(m, np.float32).T.astype(ml_dtypes.bfloat16)  # [in, out]
        # wts[i, j, p, k*128+m] = M.T[k*128+p, j*128+m]
        wts[i] = mt.reshape(nk, 128, nh, 128).transpose(2, 1, 0, 3).reshape(
            nh, 128, nk * 128
        )

    b_r = np.asarray(inputs["b_Wr"], np.float32) + np.asarray(inputs["b_Ur"], np.float32)
    b_u = np.asarray(inputs["b_Wu"], np.float32) + np.asarray(inputs["b_Uu"], np.float32)
    b_c = np.asarray(inputs["b_W"], np.float32) + np.asarray(inputs["b_U"], np.float32)
    bias = np.concatenate(
        [bb.reshape(nh, 128).T for bb in (b_r, b_u, b_c)], axis=1
    ).astype(np.float32)  # [128, 3*nh]

    in_maps = []
    for s in range(n_shards):
        rows = slice(s * b_shard, (s + 1) * b_shard)
        xT = np.ascontiguousarray(x[rows].T).astype(ml_dtypes.bfloat16)
        hT = np.ascontiguousarray(h[rows].T).astype(ml_dtypes.bfloat16)
        in_maps.append({"xt": xT, "ht": hT, "wts": wts, "bias": bias})
    return in_maps


_NC_CACHE = {}


def _get_nc():
    if "nc" not in _NC_CACHE:
        _NC_CACHE["nc"] = build_nc()
    return _NC_CACHE["nc"]


def _run(inputs, **spmd_kwargs):
    nc = _get_nc()
    in_maps = pack_inputs(inputs)
    res = run_bass_kernel_spmd(nc, in_maps, list(range(N_CORES)), **spmd_kwargs)
    out = np.empty((B, D), np.float32)
    for c in range(N_CORES):
        out[c * B_SHARD : (c + 1) * B_SHARD, :] = (
            res.results[c]["out"].astype(np.float32).T
        )
    return out, [res]


def kernel(**inputs):
    out, _ = _run(inputs)
    return out


# revision 6
# speedup vs baseline: 1.2048x; 1.0144x over previous
"""GRU cell kernel for Trainium2, 8-core data-parallel, single dispatch.

Strategy
--------
Data-parallel on batch across 8 cores; each core processes its full
2048-row shard in ONE NEFF dispatch.  All on-chip compute happens in
*transposed space* ([hidden, batch]) so every matmul contraction lands
on SBUF partitions with no on-device transposes:

    r^T = sigmoid(W_r @ x^T + U_r @ h^T + b_r)
    u^T = sigmoid(W_u @ x^T + U_u @ h^T + b_u)
    c^T = tanh   (W   @ x^T + U  @ (h.r)^T + b_c)
    o^T = h^T + u^T * (c^T - h^T)

Matmuls run in bf16.  The 2048-wide batch gives 4 moving slices of 512
per stationary weight tile, which keeps the tensor engine at its
~216 ns/MM back-to-back cadence (weight-swap drain stalls amortize).

DMA conveyor: the HWDGE rings dispatch ~1 DMA instruction per ~600 ns
regardless of size, so per-128x128-tile weight loads (384 of them)
rate-limit the whole kernel (measured: PE catches the weight stream
and idles 47 us).  v3 loads weights as 48 contiguous 256 KiB slabs
(one per (matrix, output-tile), host-side packed) on the Sync ring,
and puts x on the Scalar ring so the two conveyors run in parallel.
The R phase consumes k-tiles in arrival order.

SBUF budget (per partition, ~208 KiB usable): weights 96K + x 32K +
h 32K + hr 16K + pools ~20K.  hr is stored fp8_e4m3 (moving operand
only) and the output chain runs bf16 in-place in the c tile; measured
end-to-end rel err 7.5e-3 vs the 2e-2 budget.

DMA rules: descriptors encode exactly ONE sync wait, so no load DMA
may target a recycled tile slot (loads carry only queue-FIFO waits ->
every DMA'd tile gets its own slot), and the 8 output stores ride 8
distinct SWDGE queues so their single RAW wait fits.
"""

import sys

sys.path.insert(0, "/opt/trn_rl_repo")

import numpy as np
import ml_dtypes
from contextlib import ExitStack

import concourse.bass as bass
import concourse.bacc as bacc
import concourse.mybir as mybir
from concourse import tile
from concourse.bass_utils import run_bass_kernel_spmd

BF16 = mybir.dt.bfloat16
FP8 = mybir.dt.float8e4
F32 = mybir.dt.float32
AF = mybir.ActivationFunctionType

N_CORES = 8
B = 16384
D = 1024  # IN == H
B_SHARD = B // N_CORES  # 2048 rows per core, single dispatch
BW = 512  # matmul moving width (one fp32 PSUM bank)

# k-tile consumption order: ascending matches the conveyor arrival order
KORD = list(range(8))


def build_nc(d=D, b_shard=B_SHARD, bw=BW):
    """Build the SPMD per-core Bass program.

    Packed weight order: 0=W_r, 1=U_r, 2=W_u, 3=U_u, 4=W, 5=U.
    Bias columns: [r: 0..nh) [u: nh..2nh) [c: 2nh..3nh).
    """
    nk = d // 128
    nh = d // 128
    nb = b_shard // bw

    nc = bacc.Bacc("TRN2", target_bir_lowering=False)
    xt = nc.dram_tensor("xt", [d, b_shard], BF16, kind="ExternalInput")
    ht = nc.dram_tensor("ht", [d, b_shard], BF16, kind="ExternalInput")
    # weight slab (mat, j): [128 partitions, nk*128] contiguous
    wts = nc.dram_tensor("wts", [6, nh, 128, nk * 128], BF16, kind="ExternalInput")
    bias = nc.dram_tensor("bias", [128, 3 * nh], F32, kind="ExternalInput")
    out = nc.dram_tensor("out", [d, b_shard], BF16, kind="ExternalOutput")

    with tile.TileContext(nc) as tc, ExitStack() as ctx:
        xp = ctx.enter_context(tc.tile_pool(name="xp", bufs=nk))
        hp = ctx.enter_context(tc.tile_pool(name="hp", bufs=nk))
        hrp = ctx.enter_context(tc.tile_pool(name="hrp", bufs=nh))
        rp = ctx.enter_context(tc.tile_pool(name="rp", bufs=2))
        up = ctx.enter_context(tc.tile_pool(name="up", bufs=2))
        cp = ctx.enter_context(tc.tile_pool(name="cp", bufs=3))
        # every weight slab gets its own slot: no DMA slot reuse anywhere
        wp = ctx.enter_context(tc.tile_pool(name="wp", bufs=6 * nh))
        bp = ctx.enter_context(tc.tile_pool(name="bp", bufs=1))
        pp = ctx.enter_context(tc.tile_pool(name="pp", bufs=8, space="PSUM"))

        wslabs = {}

        def load_w(mat, j):
            t = wp.tile([128, nk * 128], BF16, name="wslab")
            nc.sync.dma_start(t, wts[mat, j, :, :])
            wslabs[(mat, j)] = t

        xts, hts = [None] * nk, [None] * nk
        half = b_shard // 2

        # Half-tile loads: the conveyor is HBM-fair-share bound at startup,
        # so finer DMA granularity gets the first k-tiles consumable sooner.
        def load_x(k):
            xts[k] = xp.tile([128, b_shard], BF16, name="xtile")
            nc.scalar.dma_start(xts[k][:, :half], xt[k * 128 : (k + 1) * 128, :half])
            nc.scalar.dma_start(xts[k][:, half:], xt[k * 128 : (k + 1) * 128, half:])

        def load_h(k):
            hts[k] = hp.tile([128, b_shard], BF16, name="htile")
            nc.sync.dma_start(hts[k][:, :half], ht[k * 128 : (k + 1) * 128, :half])
            nc.sync.dma_start(hts[k][:, half:], ht[k * 128 : (k + 1) * 128, half:])

        # scalar ring: all of x, then bias.  sync ring: gate-0 R slabs, h
        # interleaved with the later R slabs, then UC slabs in use order.
        for k in range(nk):
            load_x(k)
        btile = bp.tile([128, 3 * nh], F32, name="btile")
        nc.scalar.dma_start(btile, bias[:, :])
        load_w(0, 0)
        load_w(1, 0)
        load_h(0)
        load_h(1)
        load_w(0, 1)
        load_w(1, 1)
        load_h(2)
        load_h(3)
        load_w(0, 2)
        load_w(1, 2)
        load_h(4)
        load_h(5)
        load_w(0, 3)
        load_w(1, 3)
        load_h(6)
        load_h(7)
        for j in range(4, nh):
            load_w(0, j)
            load_w(1, j)
        for j in range(nh):
            for mat in (2, 3, 4, 5):
                load_w(mat, j)

        def gate_matmuls(j, mat_x, mov_x, mat_h, mov_h):
            """Accumulate x-part + h-part for gate tile j into nb PSUM banks."""
            ps = [pp.tile([128, bw], F32, name="ps") for _ in range(nb)]
            for mi, (mat, mov) in enumerate(((mat_x, mov_x), (mat_h, mov_h))):
                slab = wslabs[(mat, j)]
                for ki, k in enumerate(KORD):
                    lhsT = slab[:, k * 128 : (k + 1) * 128]
                    for b in range(nb):
                        nc.tensor.matmul(
                            ps[b],
                            lhsT,
                            mov[k][:, b * bw : (b + 1) * bw],
                            start=(mi == 0 and ki == 0),
                            stop=(mi == 1 and ki == nk - 1),
                        )
            return ps

        # R phase: r = sigmoid(...); hr = h * r in fp8 (feeds the c matmuls)
        hrs = []
        for j in range(nh):
            ps = gate_matmuls(j, 0, xts, 1, hts)
            rtile = rp.tile([128, b_shard], BF16, name="rtile")
            for b in range(nb):
                nc.scalar.activation(
                    rtile[:, b * bw : (b + 1) * bw], ps[b], AF.Sigmoid,
                    bias=btile[:, j : j + 1],
                )
            hrtile = hrp.tile([128, b_shard], FP8, name="hrtile")
            nc.vector.tensor_mul(hrtile, hts[j], rtile)
            hrs.append(hrtile)

        # U+C fused per j, out chain in-place in ctile, chunked per bank so
        # the tail (ACT -> DVE -> store) pipelines at 512 granularity.
        for j in range(nh):
            ps = gate_matmuls(j, 2, xts, 3, hts)
            util = up.tile([128, b_shard], BF16, name="utile")
            for b in range(nb):
                nc.scalar.activation(
                    util[:, b * bw : (b + 1) * bw], ps[b], AF.Sigmoid,
                    bias=btile[:, nh + j : nh + j + 1],
                )
            ps = gate_matmuls(j, 4, xts, 5, hrs)
            ctile = cp.tile([128, b_shard], BF16, name="ctile")
            for b in range(nb):
                s = slice(b * bw, (b + 1) * bw)
                nc.scalar.activation(
                    ctile[:, s], ps[b], AF.Tanh,
                    bias=btile[:, 2 * nh + j : 2 * nh + j + 1],
                )
                # o = h + u*(c - h), computed in place in ctile (bf16)
                nc.vector.tensor_sub(ctile[:, s], ctile[:, s], hts[j][:, s])
                nc.vector.tensor_mul(ctile[:, s], util[:, s], ctile[:, s])
                nc.vector.tensor_add(ctile[:, s], ctile[:, s], hts[j][:, s])
                # per-bank store on the sync HWDGE ring (idle after loads):
                # pipelines the tail instead of one big end-of-gate store.
                nc.sync.dma_start(out[j * 128 : (j + 1) * 128, s], ctile[:, s])

    # Bacc lowering: splits multi-wait sync into InstEventSemaphore ops
    # (hardware allows one wait per instruction), allocates registers, etc.
    nc.compile()
    return nc


def pack_inputs(inputs, d=D, b_shard=B_SHARD, n_shards=N_CORES):
    """Host-side shard + transpose + cast. Returns per-shard input maps."""
    nk = d // 128
    nh = d // 128
    x = np.asarray(inputs["x_t"], np.float32)
    h = np.asarray(inputs["h_prev"], np.float32)

    mats = [inputs["W_r"], inputs["U_r"], inputs["W_u"], inputs["U_u"],
            inputs["W"], inputs["U"]]
    wts = np.empty((6, nh, 128, nk * 128), ml_dtypes.bfloat16)
    for i, m in enumerate(mats):
        mt = np.asarray(m, np.float32).T.astype(ml_dtypes.bfloat16)  # [in, out]
        # wts[i, j, p, k*128+m] = M.T[k*128+p, j*128+m]
        wts[i] = mt.reshape(nk, 128, nh, 128).transpose(2, 1, 0, 3).reshape(
            nh, 128, nk * 128
        )

    b_r = np.asarray(inputs["b_Wr"], np.float32) + np.asarray(inputs["b_Ur"], np.float32)
    b_u = np.asarray(inputs["b_Wu"], np.float32) + np.asarray(inputs["b_Uu"], np.float32)
    b_c = np.asarray(inputs["b_W"], np.float32) + np.asarray(inputs["b_U"], np.float32)
    bias = np.concatenate(
        [bb.reshape(nh, 128).T for bb in (b_r, b_u, b_c)], axis=1
    ).astype(np.float32)  # [128, 3*nh]

    in_maps = []
    for s in range(n_shards):
        rows = slice(s * b_shard, (s + 1) * b_shard)
        xT = np.ascontiguousarray(x[rows].T).astype(ml_dtypes.bfloat16)
        hT = np.ascontiguousarray(h[rows].T).astype(ml_dtypes.bfloat16)
        in_maps.append({"xt": xT, "ht": hT, "wts": wts, "bias": bias})
    return in_maps


_NC_CACHE = {}


def _get_nc():
    if "nc" not in _NC_CACHE:
        _NC_CACHE["nc"] = build_nc()
    return _NC_CACHE["nc"]


def _run(inputs, **spmd_kwargs):
    nc = _get_nc()
    in_maps = pack_inputs(inputs)
    res = run_bass_kernel_spmd(nc, in_maps, list(range(N_CORES)), **spmd_kwargs)
    out = np.empty((B, D), np.float32)
    for c in range(N_CORES):
        out[c * B_SHARD : (c + 1) * B_SHARD, :] = (
            res.results[c]["out"].astype(np.float32).T
        )
    return out, [res]


def kernel(**inputs):
    out, _ = _run(inputs)
    return out
